# revision 38
# baseline (speedup 1.0000x reference)
"""GraphTransformerLayer — full-device Trainium2 kernel (8 NeuronCores).

Distribution (per the dst-partitioned sharding strategy):
  * Nodes are degree-sorted on host and grouped into 128-node chunks so each
    chunk has a tight max in-degree T.  Chunks are dealt round-robin to the 8
    cores (load balance) and every core runs the SAME program (shared
    per-chunk-slot T schedule = max over cores).
  * Each core computes Q/K/V/skip projections for its 2560-node shard on the
    TensorEngine (bf16), the K|V rows are AllGathered so every core holds the
    full [20480, 512] bf16 K|V table in DRAM.
  * Edge stage is destination-partitioned: per chunk one dma_gather pulls the
    chunk's source K|V rows into a [128 dst, T, 512] slot layout; scores,
    segment-softmax and the alpha-weighted aggregation run on DVE/ACT with
    every bulk op in bf16 2x mode.  V is stored head-minor (d*4+h) so the
    p-broadcast multiply keeps unit inner stride.
  * BatchNorm batch stats go through a ones-vector matmul + AllReduce; the
    affine + exact GELU epilogue runs on device.  Host only shards inputs,
    builds index/mask tables and un-permutes the output rows.
"""
import math

import numpy as np

try:
    from ml_dtypes import bfloat16 as _bf16
except Exception:  # pragma: no cover
    _bf16 = None

N = 20000
E_EXPECT = 320000
IN = 128
D = 64
H = 4
HD = H * D          # 256
KVW = 2 * HD        # 512  (k row | v row)
WCOLS = 3 * HD + D  # 832
EPS_BN = 1e-5
N_CORES = 8
CHUNK = 128
NCH = 20                      # chunks per core
PER_CORE = NCH * CHUNK        # 2560
NPAD = N_CORES * PER_CORE     # 20480
MASK_NEG = -30000.0
TCAP = 20


# ----------------------------------------------------------------------------
# Host-side preprocessing: degree-sorted node permutation, per-chunk slot
# tables, gather indices and masks.
# ----------------------------------------------------------------------------

def _preprocess(edge_index):
    src = np.asarray(edge_index[0], np.int64)
    dst = np.asarray(edge_index[1], np.int64)
    E = src.shape[0]

    deg = np.bincount(dst, minlength=NPAD).astype(np.int64)  # phantoms deg 0
    order = np.argsort(deg, kind="stable")                   # ascending degree
    # sorted position p -> device row (core-block layout for AllGather)
    p = np.arange(NPAD)
    core_of_p = (p // CHUNK) % N_CORES
    k_of_p = (p // CHUNK) // N_CORES
    devrow_of_p = core_of_p * PER_CORE + k_of_p * CHUNK + (p % CHUNK)
    dev = np.empty(NPAD, np.int64)
    dev[order] = devrow_of_p

    # per-sorted-chunk max degree -> shared per-slot T across cores
    chunk_max = deg[order].reshape(NPAD // CHUNK, CHUNK).max(axis=1)
    Ts = np.maximum(chunk_max.reshape(NCH, N_CORES).max(axis=1), 1).astype(int)
    Toff = np.zeros(NCH + 1, np.int64)
    Toff[1:] = np.cumsum(Ts)
    TSUM = int(Toff[-1])

    dd = dev[dst]
    ss = dev[src]
    # slot index of each edge within its destination row
    eorder = np.argsort(dd, kind="stable")
    sdd = dd[eorder]
    counts = np.bincount(sdd, minlength=NPAD)
    starts = np.zeros(NPAD, np.int64)
    starts[1:] = np.cumsum(counts)[:-1]
    slot_sorted = np.arange(E) - starts[sdd]
    slot = np.empty(E, np.int64)
    slot[eorder] = slot_sorted

    c_e = dd // PER_CORE
    within = dd % PER_CORE
    k_e = within // CHUNK
    dloc_e = within % CHUNK

    # flat slot-major gather index per core: pos = (Toff[k]+t)*128 + dloc
    idx_flat = np.zeros((N_CORES, TSUM * CHUNK), np.int64)
    val_flat = np.zeros((N_CORES, TSUM * CHUNK), bool)
    pos = (Toff[k_e] + slot) * CHUNK + dloc_e
    idx_flat[c_e, pos] = ss
    val_flat[c_e, pos] = True

    assert idx_flat.max() < 32768
    # wrapped int16 layout: index i -> [i % 16, i // 16], replicated to 128 p
    wrapped = idx_flat.reshape(N_CORES, TSUM * 8, 16).transpose(0, 2, 1)
    idx16 = np.tile(wrapped.astype(np.int16), (1, 8, 1))  # [8, 128, TSUM*8]

    # additive score mask [core, 128, TSUM, H]
    vmask = val_flat.reshape(N_CORES, TSUM, CHUNK).transpose(0, 2, 1)
    maskb = np.where(vmask[..., None], 0.0, MASK_NEG).astype(np.float32)
    maskb = np.repeat(maskb, H, axis=2).reshape(N_CORES, CHUNK, TSUM * H)

    # per-node validity (real node vs phantom pad), [core, 128, NCH]
    is_real = (order < N)
    node_valid = np.zeros(NPAD, np.float32)
    node_valid[devrow_of_p] = is_real.astype(np.float32)
    valid = node_valid.reshape(N_CORES, NCH, CHUNK).transpose(0, 2, 1).copy()

    return dev, Ts, idx16, maskb, valid


# ----------------------------------------------------------------------------
# Device program
# ----------------------------------------------------------------------------

def _tree_levels(n):
    """Pair-fold schedule: sum n slots -> slot 0. Yields (h, f): add
    [h, h+f) onto [0, f); remaining size becomes h."""
    out = []
    while n > 1:
        h = (n + 1) // 2
        f = n // 2
        out.append((h, f))
        n = h
    return out


def _build_program(Ts, n_edge=None, do_ag=True, do_edge=True, do_bn=True):
    import os as _os
    n_edge = int(_os.environ.get("GT_NEDGE", NCH if n_edge is None else n_edge))
    do_ag = bool(int(_os.environ.get("GT_AG", int(do_ag))))
    do_edge = bool(int(_os.environ.get("GT_EDGE", int(do_edge))))
    do_bn = bool(int(_os.environ.get("GT_BN", int(do_bn))))
    import concourse.bass as bass
    import concourse.tile as tile
    from concourse import bacc, mybir

    f32 = mybir.dt.float32
    bf16 = mybir.dt.bfloat16
    i16 = mybir.dt.int16

    Ts = [int(t) for t in Ts]
    Toff = [0]
    for t in Ts:
        Toff.append(Toff[-1] + t)
    TSUM = Toff[-1]
    TMAX = max(Ts)

    nc = bacc.Bacc("TRN2", target_bir_lowering=False, debug=False,
                   num_devices=N_CORES, num_swdge_queues=4,
                   dynamic_dma_scratch_size=32768)

    xT_d = nc.dram_tensor("xT", [IN, PER_CORE], bf16, kind="ExternalInput")
    W_d = nc.dram_tensor("W", [IN, WCOLS], bf16, kind="ExternalInput")
    idx_d = nc.dram_tensor("IDX", [CHUNK, TSUM * 8], i16, kind="ExternalInput")
    mask_d = nc.dram_tensor("MASKB", [CHUNK, TSUM * H], bf16,
                            kind="ExternalInput")
    valid_d = nc.dram_tensor("VALID", [CHUNK, NCH], f32, kind="ExternalInput")
    gb_d = nc.dram_tensor("GB", [1, 2 * D], f32, kind="ExternalInput")
    out_d = nc.dram_tensor("OUT", [PER_CORE, D], f32, kind="ExternalOutput")

    kv_mine = nc.dram_tensor("KVmine", [PER_CORE, KVW], bf16)
    kv_all = nc.dram_tensor("KVall", [NPAD, KVW], bf16, addr_space="Shared")
    bn_in = nc.dram_tensor("BNin", [1, 2 * D], f32)
    bn_out = nc.dram_tensor("BNout", [1, 2 * D], f32, addr_space="Shared")

    groups = [list(range(N_CORES))]

    with tile.TileContext(nc) as tc:
        with (
            tc.tile_pool(name="const", bufs=1) as constp,
            tc.tile_pool(name="proj", bufs=4) as projp,
            tc.tile_pool(name="psum", bufs=2, space="PSUM") as psump,
            tc.tile_pool(name="gath", bufs=4) as gathp,
            tc.tile_pool(name="work", bufs=3) as workp,
            tc.tile_pool(name="small", bufs=4) as smallp,
            tc.tile_pool(name="fin", bufs=1) as finp,
        ):
            # ---- constants / persistent SBUF ----
            xT = constp.tile([IN, PER_CORE], bf16, tag="xT")
            nc.sync.dma_start(xT[:], xT_d[:, :])
            Wsb = constp.tile([IN, WCOLS], bf16, tag="W")
            nc.sync.dma_start(Wsb[:], W_d[:, :])
            idx_sb = constp.tile([CHUNK, TSUM * 8], i16, tag="idx")
            nc.sync.dma_start(idx_sb[:], idx_d[:, :])
            mask_sb = constp.tile([CHUNK, TSUM * H], bf16, tag="mask")
            nc.sync.dma_start(mask_sb[:], mask_d[:, :])
            valid_sb = constp.tile([CHUNK, NCH], f32, tag="valid")
            nc.sync.dma_start(valid_sb[:], valid_d[:, :])
            gb_sb = constp.tile([1, 2 * D], f32, tag="gb")
            nc.sync.dma_start(gb_sb[:], gb_d[:, :])
            ones1 = constp.tile([1, CHUNK], bf16, tag="ones1")
            nc.vector.memset(ones1[:], 1.0)
            onesP = constp.tile([CHUNK, 1], bf16, tag="onesP")
            nc.vector.memset(onesP[:], 1.0)
            epsH = constp.tile([CHUNK, H], f32, tag="epsH")
            nc.vector.memset(epsH[:], 1e-16)
            invN = constp.tile([1, D], f32, tag="invN")
            nc.vector.memset(invN[:], 1.0 / N)
            epsD = constp.tile([1, D], f32, tag="epsD")
            nc.vector.memset(epsD[:], EPS_BN)

            Qall = constp.tile([CHUNK, NCH, HD], bf16, tag="Qall")
            OUTS = constp.tile([CHUNK, NCH, D], f32, tag="OUTS")

            # ---- Phase A: K|V projections first (AG depends on them) ----
            for k in range(NCH):
                lhs = xT[:, k * CHUNK:(k + 1) * CHUNK]
                ps_kv = psump.tile([CHUNK, KVW], f32, tag="pskv")
                nc.tensor.matmul(ps_kv[:], lhs, Wsb[:, 0:KVW],
                                 start=True, stop=True)
                kv_sb = projp.tile([CHUNK, KVW], bf16, tag="kvsb")
                nc.scalar.copy(kv_sb[:], ps_kv[:])
                nc.sync.dma_start(kv_mine[k * CHUNK:(k + 1) * CHUNK, :],
                                  kv_sb[:])

            # ---- Phase B: AllGather K|V ----
            if do_ag:
                nc.gpsimd.collective_compute(
                    "AllGather", mybir.AluOpType.bypass,
                    replica_groups=groups,
                    ins=[kv_mine.ap().opt()], outs=[kv_all.ap().opt()])

            # ---- Phase A2: Q|skip projections (overlap the AllGather) ----
            for k in range(NCH):
                lhs = xT[:, k * CHUNK:(k + 1) * CHUNK]
                ps_qs = psump.tile([CHUNK, HD + D], f32, tag="psqs")
                nc.tensor.matmul(ps_qs[:], lhs, Wsb[:, KVW:WCOLS],
                                 start=True, stop=True)
                nc.scalar.copy(Qall[:, k, :], ps_qs[:, 0:HD])
                # skip -> persistent f32 (msg is added on top in phase C)
                nc.scalar.copy(OUTS[:, k, :], ps_qs[:, HD:HD + D])

            # ---- Phase C: edge pieces (partial softmax per piece) ----
            # piece schedule: split chunks to <= TCAP slots, big first,
            # pieces of one chunk stay adjacent
            groups_p = []
            for k in range(n_edge):
                T = Ts[k]
                npc = (T + TCAP - 1) // TCAP
                bp = [round(j * T / npc) for j in range(npc + 1)]
                groups_p.append([(k, bp[j], bp[j + 1]) for j in range(npc)])
            groups_p.sort(key=lambda g: -max(r - l for (_, l, r) in g))

            macc = constp.tile([CHUNK, D, H], f32, tag="macc")
            dacc = constp.tile([CHUNK, H], f32, tag="dacc")

            gath_cnt = 0
            for grp in groups_p:
                npc = len(grp)
                for (pi, (k, pl, pr)) in enumerate(grp):
                    T = pr - pl
                    kvg = gathp.tile([CHUNK, TCAP, KVW], bf16, tag="kvg")
                    bnds = sorted(set(round(j * T / 4) for j in range(5)))
                    for (l, r) in zip(bnds[:-1], bnds[1:]):
                        # Tile round-robins SWDGE DMA-completion sems over
                        # 8 lanes; keep queue == lane % 4 so each sem stays
                        # bound to one queue (ucode requirement).
                        nc.gpsimd.dma_gather(
                            out_ap=kvg[:, l:r, :],
                            in_ap=kv_all.ap(),
                            idxs_ap=idx_sb[:, (Toff[k] + pl + l) * 8:
                                           (Toff[k] + pl + r) * 8],
                            num_idxs=(r - l) * CHUNK,
                            num_idxs_reg=(r - l) * CHUNK,
                            elem_size=KVW,
                            single_packet=False,
                            queue_num=(gath_cnt % 8) % 4,
                        )
                        gath_cnt += 1
                    if not do_edge:
                        continue
                    kg = kvg[:, 0:T, 0:HD]                  # [128, T, 256]
                    vg = kvg[:, 0:T, HD:KVW]                # [128, T, 256]

                    # scores: qk = kg * q (broadcast over T), tree over d
                    qk = workp.tile([CHUNK, TCAP, HD], bf16, tag="qk")
                    qb = Qall[:, k:k + 1, :].broadcast_to([CHUNK, T, HD])
                    nc.vector.tensor_mul(qk[:, 0:T, :], kg, qb)
                    qk4 = qk[:, 0:T, :].rearrange("p t (h w) -> p t h w",
                                                  h=H)
                    for (h, f) in _tree_levels(D):
                        nc.vector.tensor_add(qk4[:, :, :, 0:f],
                                             qk4[:, :, :, 0:f],
                                             qk4[:, :, :, h:h + f])
                    # masked scores, compact [128, T, H]
                    sbuf = smallp.tile([CHUNK, TCAP, H], bf16, tag="sc")
                    mo = (Toff[k] + pl) * H
                    mk = mask_sb[:, mo:mo + T * H] \
                        .rearrange("p (t h) -> p t h", h=H)
                    nc.vector.tensor_add(sbuf[:, 0:T, :], qk4[:, :, :, 0],
                                         mk)
                    # exp (scale 1/sqrt(D)) + fused per-head denominators
                    p_sb = smallp.tile([CHUNK, TCAP, H], bf16, tag="p")
                    den = smallp.tile([CHUNK, H], f32, tag="den")
                    s3 = sbuf[:, 0:T, :]
                    for h in range(H):
                        nc.scalar.activation(
                            p_sb[:, 0:T, h], s3[:, :, h],
                            mybir.ActivationFunctionType.Exp,
                            bias=0.0, scale=1.0 / math.sqrt(D),
                            accum_out=den[:, h:h + 1])

                    # alpha*v (in place over qk); v is head-minor
                    vg4 = vg.rearrange("p t (w h) -> p t w h", h=H)
                    pb = p_sb[:, 0:T, :].unsqueeze(2) \
                        .broadcast_to([CHUNK, T, D, H])
                    av4 = qk[:, 0:T, :].rearrange("p t (w h) -> p t w h",
                                                  h=H)
                    nc.vector.tensor_mul(av4[:], vg4, pb)
                    avt = qk[:, 0:T, :]
                    for (h, f) in _tree_levels(T):
                        nc.vector.tensor_add(avt[:, 0:f, :],
                                             avt[:, 0:f, :],
                                             avt[:, h:h + f, :])
                    m4p = qk[:, 0, :].rearrange("p (w h) -> p w h", h=H)

                    if npc > 1:
                        if pi == 0:
                            nc.vector.tensor_copy(macc[:], m4p)
                            nc.vector.tensor_copy(dacc[:], den[:])
                        else:
                            nc.vector.tensor_add(macc[:], macc[:], m4p)
                            nc.vector.tensor_add(dacc[:], dacc[:], den[:])
                        if pi < npc - 1:
                            continue
                        m4, dfin = macc[:], dacc
                    else:
                        m4, dfin = m4p, den

                    # recip(denom + eps); 1/H folded into Wv on host
                    rden = smallp.tile([CHUNK, H], f32, tag="rden")
                    nc.vector.tensor_add(dfin[:], dfin[:], epsH[:])
                    nc.vector.reciprocal(rden[:], dfin[:])
                    # normalize, mean over heads, add skip, valid-mask
                    rb = rden[:].unsqueeze(1).broadcast_to([CHUNK, D, H])
                    mm = smallp.tile([CHUNK, D, H], f32, tag="mm")
                    nc.vector.tensor_mul(mm[:], m4, rb)
                    for (h, f) in _tree_levels(H):
                        nc.vector.tensor_add(mm[:, :, 0:f], mm[:, :, 0:f],
                                             mm[:, :, h:h + f])
                    nc.vector.tensor_add(OUTS[:, k, :], mm[:, :, 0],
                                         OUTS[:, k, :])
                    vb = valid_sb[:, k:k + 1].broadcast_to([CHUNK, D])
                    nc.vector.tensor_mul(OUTS[:, k, :], OUTS[:, k, :], vb)

            # ---- Phase D: BatchNorm stats + AllReduce + affine + GELU ----
            if not do_bn:
                od = out_d.ap().rearrange("(k p) d -> p k d", p=CHUNK)
                nc.sync.dma_start(od, OUTS[:])
                nc.compile()
                return nc
            sq = finp.tile([CHUNK, NCH, D], f32, tag="sqoutf")
            nc.scalar.square(sq[:], OUTS[:])
            red = finp.tile([CHUNK, 2, D], f32, tag="red")
            lv = _tree_levels(NCH)
            (h0, f0) = lv[0]
            # first fold into scratch, rest in place
            sc0 = finp.tile([CHUNK, (NCH + 1) // 2, D], f32, tag="sc0")
            sc1 = finp.tile([CHUNK, (NCH + 1) // 2, D], f32, tag="sc1")
            nc.vector.tensor_add(sc0[:, 0:f0, :], OUTS[:, 0:f0, :],
                                 OUTS[:, h0:h0 + f0, :])
            if h0 > f0:
                nc.vector.tensor_copy(sc0[:, f0:h0, :], OUTS[:, f0:h0, :])
            nc.vector.tensor_add(sc1[:, 0:f0, :], sq[:, 0:f0, :],
                                 sq[:, h0:h0 + f0, :])
            if h0 > f0:
                nc.vector.tensor_copy(sc1[:, f0:h0, :], sq[:, f0:h0, :])
            for (h, f) in lv[1:]:
                nc.vector.tensor_add(sc0[:, 0:f, :], sc0[:, 0:f, :],
                                     sc0[:, h:h + f, :])
                nc.vector.tensor_add(sc1[:, 0:f, :], sc1[:, 0:f, :],
                                     sc1[:, h:h + f, :])
            nc.vector.tensor_copy(red[:, 0, :], sc0[:, 0, :])
            nc.vector.tensor_copy(red[:, 1, :], sc1[:, 0, :])
            # partition reduction via ones-matmul -> [1, 128]
            cat_b = finp.tile([CHUNK, 2 * D], bf16, tag="catb")
            nc.vector.tensor_copy(cat_b[:], red[:].rearrange(
                "p a d -> p (a d)"))
            ps_bn = psump.tile([1, 2 * D], f32, tag="psbn")
            nc.tensor.matmul(ps_bn[:], onesP[:], cat_b[:],
                             start=True, stop=True)
            bn_sb = smallp.tile([1, 2 * D], f32, tag="bnsb")
            nc.vector.tensor_copy(bn_sb[:], ps_bn[:])
            nc.sync.dma_start(bn_in[:, :], bn_sb[:])
            nc.gpsimd.collective_compute(
                "AllReduce", mybir.AluOpType.add, replica_groups=groups,
                ins=[bn_in.ap().opt()], outs=[bn_out.ap().opt()])
            agg = smallp.tile([1, 2 * D], f32, tag="agg")
            nc.sync.dma_start(agg[:], bn_out[:, :])
            # mu, var, g = gamma*rsqrt(var+eps), b = beta - mu*g  (1 part.)
            mu = smallp.tile([1, D], f32, tag="mu")
            nc.vector.tensor_mul(mu[:], agg[:, 0:D], invN[:])
            ex2 = smallp.tile([1, D], f32, tag="ex2")
            nc.vector.tensor_mul(ex2[:], agg[:, D:2 * D], invN[:])
            var = smallp.tile([1, D], f32, tag="var")
            nc.vector.tensor_mul(var[:], mu[:], mu[:])
            nc.vector.tensor_sub(var[:], ex2[:], var[:])
            nc.vector.tensor_add(var[:], var[:], epsD[:])
            std = smallp.tile([1, D], f32, tag="std")
            nc.scalar.activation(std[:], var[:],
                                 mybir.ActivationFunctionType.Sqrt,
                                 bias=0.0, scale=1.0)
            rstd = smallp.tile([1, D], f32, tag="rstd")
            nc.vector.reciprocal(rstd[:], std[:])
            gshift = smallp.tile([1, 2 * D], f32, tag="gshift")
            nc.vector.tensor_mul(gshift[:, 0:D], gb_sb[:, 0:D], rstd[:])
            nc.vector.tensor_mul(gshift[:, D:2 * D], mu[:], gshift[:, 0:D])
            nc.vector.tensor_sub(gshift[:, D:2 * D], gb_sb[:, D:2 * D],
                                 gshift[:, D:2 * D])
            gs_b = smallp.tile([1, 2 * D], bf16, tag="gsb")
            nc.vector.tensor_copy(gs_b[:], gshift[:])
            ps_rep = psump.tile([CHUNK, 2 * D], f32, tag="psrep")
            nc.tensor.matmul(ps_rep[:], ones1[:], gs_b[:],
                             start=True, stop=True)
            grep = finp.tile([CHUNK, 2 * D], f32, tag="grep")
            nc.vector.tensor_copy(grep[:], ps_rep[:])
            # out = gelu(out*g + b)
            gbc = grep[:, 0:D].unsqueeze(1).broadcast_to([CHUNK, NCH, D])
            bbc = grep[:, D:2 * D].unsqueeze(1).broadcast_to([CHUNK, NCH, D])
            nc.vector.tensor_mul(OUTS[:], OUTS[:], gbc)
            nc.vector.tensor_add(OUTS[:], OUTS[:], bbc)
            outf = finp.tile([CHUNK, NCH, D], f32, tag="sqoutf")
            nc.scalar.activation(outf[:], OUTS[:],
                                 mybir.ActivationFunctionType.Gelu,
                                 bias=0.0, scale=1.0)
            # store: device row = k*128 + dloc
            od = out_d.ap().rearrange("(k p) d -> p k d", p=CHUNK)
            nc.sync.dma_start(od, outf[:])

    nc.compile()
    return nc


# ----------------------------------------------------------------------------
# kernel entry
# ----------------------------------------------------------------------------

def _to_bf16(a):
    return np.asarray(a, np.float32).astype(_bf16)


def _make_inputs(x, edge_index, Wq, bq, Wk, bk, Wv, bv, Wskip, bskip,
                 gamma, beta):
    dev, Ts, idx16, maskb, valid = _preprocess(edge_index)

    # fold biases into W via nothing (biases are added post-matmul on host
    # side only if nonzero -- this problem has all-zero biases, but keep
    # correctness by folding them into the K/V/Q tables via x augmentation
    # being impossible; instead assert and fall back to host if nonzero).
    x = np.asarray(x, np.float32)
    xpad = np.zeros((NPAD, IN), np.float32)
    xpad[dev[:N]] = x

    # head-minor permutation; the 1/H of the head-mean is folded in here
    Wv_perm = np.asarray(Wv, np.float32).reshape(IN, H, D) \
        .transpose(0, 2, 1).reshape(IN, HD) * (1.0 / H)
    # layout matches the device phases: [Wk | Wv_perm] for the K|V pass,
    # then [Wq | Wskip] for the Q|skip pass
    W = np.concatenate([np.asarray(Wk, np.float32),
                        Wv_perm,
                        np.asarray(Wq, np.float32),
                        np.asarray(Wskip, np.float32)], axis=1)

    gb = np.concatenate([np.asarray(gamma, np.float32),
                         np.asarray(beta, np.float32)])[None, :]

    in_maps = []
    for c in range(N_CORES):
        xT = np.ascontiguousarray(
            xpad[c * PER_CORE:(c + 1) * PER_CORE].T).astype(_bf16)
        in_maps.append({
            "xT": xT,
            "W": W.astype(_bf16),
            "IDX": np.ascontiguousarray(idx16[c]),
            "MASKB": np.ascontiguousarray(maskb[c]).astype(_bf16),
            "VALID": np.ascontiguousarray(valid[c]),
            "GB": gb.astype(np.float32),
        })
    return dev, Ts, in_maps


_CACHE = {}


def _get_program(Ts):
    key = tuple(int(t) for t in Ts)
    if key not in _CACHE:
        _CACHE[key] = _build_program(Ts)
    return _CACHE[key]


def kernel(x, edge_index, Wq, bq, Wk, bk, Wv, bv, Wskip, bskip, gamma, beta):
    for b in (bq, bk, bv, bskip):
        if np.any(np.asarray(b)):
            return _host_kernel(x, edge_index, Wq, bq, Wk, bk, Wv, bv,
                                Wskip, bskip, gamma, beta)
    from concourse.bass_utils import run_bass_kernel_spmd

    dev, Ts, in_maps = _make_inputs(x, edge_index, Wq, bq, Wk, bk, Wv, bv,
                                    Wskip, bskip, gamma, beta)
    nc = _get_program(Ts)
    res = run_bass_kernel_spmd(nc, in_maps, list(range(N_CORES)))
    dev_out = np.concatenate(
        [res.results[c]["OUT"] for c in range(N_CORES)], axis=0)
    return np.ascontiguousarray(dev_out[dev[:N]]).astype(np.float32)


# ----------------------------------------------------------------------------
# host fallback (general-bias path / debugging)
# ----------------------------------------------------------------------------

def _host_kernel(x, edge_index, Wq, bq, Wk, bk, Wv, bv, Wskip, bskip,
                 gamma, beta):
    from scipy.special import erf

    x = np.asarray(x, np.float32)
    src = np.asarray(edge_index[0], np.int64)
    dst = np.asarray(edge_index[1], np.int64)
    q = (x @ Wq + bq).reshape(N, H, D)
    k = (x @ Wk + bk).reshape(N, H, D)
    v = (x @ Wv + bv).reshape(N, H, D)
    order = np.argsort(dst, kind="stable")
    s_src, s_dst = src[order], dst[order]
    scores = np.einsum("ehd,ehd->eh", q[s_dst], k[s_src],
                       dtype=np.float32) / np.float32(math.sqrt(D))
    seg_starts = np.flatnonzero(np.r_[True, s_dst[1:] != s_dst[:-1]])
    seg_ids = s_dst[seg_starts]
    smax = np.zeros((N, H), np.float32)
    smax[seg_ids] = np.maximum.reduceat(scores, seg_starts, axis=0)
    p = np.exp(scores - smax[s_dst])
    denom = np.zeros((N, H), np.float32)
    denom[seg_ids] = np.add.reduceat(p, seg_starts, axis=0)
    alpha = p / (denom[s_dst] + np.float32(1e-16))
    weighted = (alpha[:, :, None] * v[s_src]).reshape(len(s_src), H * D)
    msg = np.zeros((N, H * D), np.float32)
    msg[seg_ids] = np.add.reduceat(weighted, seg_starts, axis=0)
    out = msg.reshape(N, H, D).mean(axis=1) + x @ Wskip + bskip
    mu = out.mean(axis=0)
    var = out.var(axis=0)
    out = (out - mu) / np.sqrt(var + EPS_BN) * gamma + beta
    out = out.astype(np.float64)
    return (0.5 * out * (1.0 + erf(out / math.sqrt(2.0)))).astype(np.float32)
